# revision 34
# baseline (speedup 1.0000x reference)
"""Trainium2 Bass kernel for nn_MACEConvolutionLayer.

Strategy (8 NeuronCores, no collectives):
  - Edges are sharded by DESTINATION node range (1250 nodes/core), sorted and
    packed into 10 windows of 128 nodes x 1024 edge slots per core. Each core
    computes messages for its edge shard and segment-sums them into its own
    node shard via host-precomputed one-hot matmuls on the tensor engine.
  - Per-edge bilinear (radial-MLP features x embedded source scalars) and the
    per-node equivariant tensor products use a monomial scheme in TRANSPOSED
    (feature-major) layout: PE "replication matmuls" (constant selection
    matrices as lhsT) build row-replicated/tiled operand tiles in PSUM, the
    scalar engine casts them to bf16 SBUF, and the DVE forms the monomial
    products with stride-1 access patterns (2x mode). PE then contracts the
    monomial chunks against packed omega matrices (Clebsch-Gordan x TP
    weights, with channel-mixing/combination linears folded in).

Feature layout on device is kappa-major: col(l, i, u) = LOFF[l] + i*32 + u.
"""
import sys, os

sys.path.insert(0, '/opt/trn_rl_repo')

import numpy as np
import ml_dtypes

MUL = 32
DIMS = (1, 3, 5)
HID = 288
N_NODES = 10000
N_EDGES = 64000
RHID = 64
SQM = float(np.sqrt(MUL))
LOFF = [0, 32, 128]
SOFF = [0, 1, 4]
PATHS_FULL = [(0,0,0),(0,1,1),(0,2,2),(1,0,1),(1,1,0),(1,1,2),(1,2,1),(2,0,2),(2,1,1),(2,2,0),(2,2,2)]
O2_UVW = [(0,1,1),(0,2,2),(1,2,1)]
O2_UVU = [(0,0,0),(1,1,0),(1,1,2),(2,2,0),(2,2,2)]

N_CORES = 8
NODES_PER_CORE = 1250
WIN = 128
N_WIN = 10
ESLOT = 1024
E_PAD = N_WIN * ESLOT   # 10240
ET_PER_WIN = ESLOT // 128  # 8
BF = ml_dtypes.bfloat16


def cg_np():
    s2, s3, s5, s6 = map(np.sqrt, (2.0, 3.0, 5.0, 6.0))
    B = np.zeros((5, 3, 3))
    B[0, 0, 1] = B[0, 1, 0] = 1 / s2
    B[1, 1, 2] = B[1, 2, 1] = 1 / s2
    B[2] = np.diag([-1.0, -1.0, 2.0]) / s6
    B[3, 0, 2] = B[3, 2, 0] = 1 / s2
    B[4] = np.diag([1.0, -1.0, 0.0]) / s2
    C = {}
    C[(0, 0, 0)] = np.ones((1, 1, 1))
    C[(0, 1, 1)] = (np.eye(3) / s3)[None]
    C[(1, 0, 1)] = np.transpose(C[(0, 1, 1)], (1, 0, 2))
    C[(0, 2, 2)] = (np.eye(5) / s5)[None]
    C[(2, 0, 2)] = np.transpose(C[(0, 2, 2)], (1, 0, 2))
    C[(1, 1, 0)] = (np.eye(3) / s3)[:, :, None]
    C[(1, 1, 2)] = np.transpose(B, (1, 2, 0)) / s5
    C[(1, 2, 1)] = np.transpose(B, (1, 0, 2)) / s5
    C[(2, 1, 1)] = B / s5
    C[(2, 2, 0)] = (np.eye(5) / s5)[:, :, None]
    T = np.einsum('aij,bjk,cki->abc', B, B, B)
    C[(2, 2, 2)] = T / np.linalg.norm(T)
    return C

CG = cg_np()
PATH_LIST_O2 = O2_UVW + O2_UVU  # (i,j,k) in folded order


def support_pairs(path_ijk):
    d = {}
    for pi, (li, lj, lk) in enumerate(path_ijk):
        C = CG[(li, lj, lk)]
        for iloc in range(DIMS[li]):
            for jloc in range(DIMS[lj]):
                if np.any(np.abs(C[iloc, jloc, :]) > 1e-12):
                    d.setdefault(((li, iloc), (lj, jloc)), []).append((pi, iloc, jloc))
    return d


def build_mono_blocks_sym(path_ijk):
    d = support_pairs(path_ijk)
    blocks = {}
    for (I, J), lst in d.items():
        key = (min(I, J), max(I, J))
        swap = I > J
        for (pi, iloc, jloc) in lst:
            blocks.setdefault(key, []).append((pi, iloc, jloc, swap))
    return [(I, J, c) for (I, J), c in sorted(blocks.items())]


def build_mono_blocks(path_ijk):
    d = support_pairs(path_ijk)
    return [(I, J, [(pi, i, j, False) for (pi, i, j) in lst]) for (I, J), lst in sorted(d.items())]


def omega_for_block(path_ijk, weights, I, J, contribs):
    """[1024 (u-major,v-fast), 288] kappa-major outputs."""
    Om = np.zeros((MUL * MUL, HID))
    for (pi, iloc, jloc, swap) in contribs:
        li, lj, lk = path_ijk[pi]
        W = weights[pi]
        C = CG[(li, lj, lk)]
        for kap in range(DIMS[lk]):
            c = C[iloc, jloc, kap]
            if abs(c) < 1e-12:
                continue
            c0 = LOFF[lk] + kap * 32
            Wm = W if not swap else np.transpose(W, (1, 0, 2))
            Om[:, c0:c0 + 32] += c * Wm.reshape(MUL * MUL, MUL)
    return Om


# ---------------------------------------------------------------------------
# static plan: monomial blocks + emission structure (depends only on CG)
# ---------------------------------------------------------------------------

class Plan:
    pass


GAPTOL = 4

def _emissions_for_pair(mq, mm_):
    """Emissions in the device qm space: q outputs at cols [0:288) (PSUM
    bank 0), msg outputs at cols [512:800) (bank 1). mq/mm_: [1024, 288]
    bool masks (or None). Returns list over kchunk of list of (c0, c1).
    Runs never merge across the q/msg boundary."""
    out = []
    for kc in range(8):
        emis = []
        for mask, coff in ((mq, 0), (mm_, 512)):
            if mask is None:
                continue
            sub = mask[kc * 128:(kc + 1) * 128]
            used = [g for g in range(9) if np.any(sub[:, g * 32:(g + 1) * 32])]
            if not used:
                continue
            runs = [[used[0], used[0] + 1]]
            for g in used[1:]:
                if g - runs[-1][1] <= GAPTOL:
                    runs[-1][1] = g + 1
                else:
                    runs.append([g, g + 1])
            for (ga, gb) in runs:
                emis.append((coff + ga * 32, coff + gb * 32))
        out.append(emis)
    return out


def build_plan():
    p = Plan()
    p.aa_blocks = build_mono_blocks_sym(PATHS_FULL + PATH_LIST_O2)
    p.qa_blocks = build_mono_blocks(PATHS_FULL)
    n3a = len(PATHS_FULL)
    ones_a = [np.ones((MUL, MUL, MUL)) for _ in PATHS_FULL]
    ones_o2 = [np.ones((MUL, MUL, MUL)) for _ in PATH_LIST_O2]

    # per aa block: contribs split into q-part (o3a) and msg-part (o2)
    p.aa = []
    for (I, J, contribs) in p.aa_blocks:
        cq = [(pi, i, j, s) for (pi, i, j, s) in contribs if pi < n3a]
        cm = [(pi - n3a, i, j, s) for (pi, i, j, s) in contribs if pi >= n3a]
        mq = omega_for_block(PATHS_FULL, ones_a, I, J, cq) != 0 if cq else None
        mm_ = omega_for_block(PATH_LIST_O2, ones_o2, I, J, cm) != 0 if cm else None
        em = _emissions_for_pair(mq, mm_)
        p.aa.append((I, J, cq, cm, em))
    p.qa = []
    for (I, J, contribs) in p.qa_blocks:
        mm_ = omega_for_block(PATHS_FULL, ones_a, I, J, contribs) != 0
        em = _emissions_for_pair(None, mm_)
        p.qa.append((I, J, contribs, em))

    # assign omega column offsets; emission = (c0, c1, om_off) combined space
    off = 0
    p.aa_emi = []
    for (I, J, cq, cm, em) in p.aa:
        bk = []
        for kc in range(8):
            lst = []
            for (c0, c1) in em[kc]:
                lst.append((c0, c1, off))
                off += c1 - c0
            bk.append(lst)
        p.aa_emi.append(bk)
    p.qa_emi = []
    for (I, J, contribs, em) in p.qa:
        bk = []
        for kc in range(8):
            lst = []
            for (c0, c1) in em[kc]:
                lst.append((c0, c1, off))
                off += c1 - c0
            bk.append(lst)
        p.qa_emi.append(bk)
    p.totc = off
    n_emi = sum(len(l) for bk in p.aa_emi + p.qa_emi for l in bk)
    p.n_emi = n_emi
    return p


def pack_omega(plan, Wfold):
    """Fill the packed omega array [128, totc] (bf16) from folded weights.
    Emission col-ranges live in the combined 576-col space (q | msg)."""
    W3a = Wfold['o3a_w']; Wo2 = Wfold['o2_w']; W3b = Wfold['o3b_w']
    om = np.zeros((128, plan.totc), np.float32)

    def omcol(c):
        # device qm col -> Om col (q at [0:288), msg at [512:800) -> [288:576))
        return c - 224 if c >= 512 else c

    for bi, (I, J, cq, cm, em) in enumerate(plan.aa):
        Om = np.zeros((MUL * MUL, 2 * HID))
        if cq:
            Om[:, :HID] = omega_for_block(PATHS_FULL, W3a, I, J, cq)
        if cm:
            Om[:, HID:] = omega_for_block(PATH_LIST_O2, Wo2, I, J, cm)
        for kc in range(8):
            for (c0, c1, off) in plan.aa_emi[bi][kc]:
                om[:, off:off + (c1 - c0)] = Om[kc * 128:(kc + 1) * 128, omcol(c0):omcol(c0) + (c1 - c0)]
    for bi, (I, J, contribs, em) in enumerate(plan.qa):
        Om = np.zeros((MUL * MUL, 2 * HID))
        Om[:, HID:] = omega_for_block(PATHS_FULL, W3b, I, J, contribs)
        for kc in range(8):
            for (c0, c1, off) in plan.qa_emi[bi][kc]:
                om[:, off:off + (c1 - c0)] = Om[kc * 128:(kc + 1) * 128, omcol(c0):omcol(c0) + (c1 - c0)]
    return om.astype(BF)


def fold_weights(inp):
    f8 = np.float64
    mix_w = inp['mix_w'].astype(f8); comb_w = inp['comb_w'].astype(f8)
    M = np.einsum('olux,olxw->oluw', mix_w, comb_w) / MUL
    W1eff = np.einsum('lux,lxw->luw', inp['lin_o1'].astype(f8), M[0]) / SQM
    o2_w = []
    for pp, (i, j, k) in enumerate(O2_UVW):
        o2_w.append(np.einsum('uvx,xw->uvw', inp['o2_uvw'][pp].astype(f8) / MUL, M[1][k]))
    for pp, (i, j, k) in enumerate(O2_UVU):
        o2_w.append(np.einsum('uv,uw->uvw', inp['o2_uvu'][pp].astype(f8), M[1][k]) / SQM)
    o3a_w = [inp['o3a_uvw'][pp].astype(f8) / MUL for pp in range(len(PATHS_FULL))]
    o3b_w = [np.einsum('uvx,xw->uvw', inp['o3b_uvw'][pp].astype(f8) / MUL, M[2][k])
             for pp, (i, j, k) in enumerate(PATHS_FULL)]
    aw = inp['a_w'].astype(f8).reshape(RHID, 3, MUL, MUL)
    ab = inp['a_b'].astype(f8).reshape(3, MUL, MUL)
    scale = np.array([1.0 / np.sqrt(d) for d in DIMS]) / SQM
    aw = aw * scale[None, :, None, None]
    ab = ab * scale[:, None, None]
    A2 = np.transpose(aw, (0, 2, 1, 3)).reshape(RHID * MUL, 3 * MUL)
    B2 = np.transpose(ab, (1, 0, 2)).reshape(MUL, 3 * MUL)
    # c1 block-diagonal omegas per aT chunk (kappa-major rows/cols)
    omc1 = np.zeros((HID, HID))
    for l in range(3):
        for i in range(DIMS[l]):
            c = LOFF[l] + i * 32
            omc1[c:c + 32, c:c + 32] = W1eff[l]
    return dict(
        o3a_w=o3a_w, o2_w=o2_w, o3b_w=o3b_w,
        omc1=omc1, omself=inp['self_w'].astype(f8) / SQM,
        emb=inp['emb_w'].astype(f8) / SQM,
        A2=A2, B2=B2,
        r_w1=inp['r_w1'].astype(np.float32), r_b1=inp['r_b1'].astype(np.float32),
        r_w2=inp['r_w2'].astype(np.float32), r_b2=inp['r_b2'].astype(np.float32),
        r_w3=inp['r_w3'].astype(np.float32), r_b3=inp['r_b3'].astype(np.float32),
    )


def pack_edges(inp):
    src = np.asarray(inp['edge_index'][0]).astype(np.int64)
    dst = np.asarray(inp['edge_index'][1]).astype(np.int64)
    sh = np.asarray(inp['edge_sh'], dtype=np.float32)
    rad = np.asarray(inp['edge_radial_embedding'], dtype=np.float32)
    attr = np.asarray(inp['edge_attr'], dtype=np.float32)
    nf = np.asarray(inp['node_features'], dtype=np.float32)
    order = np.argsort(dst, kind='stable')
    dst_s = dst[order]
    cores = []
    for c in range(N_CORES):
        lo = c * NODES_PER_CORE
        rinT = np.zeros((24, E_PAD), BF)
        nfsT = np.zeros((MUL, E_PAD), BF)
        shv = np.zeros((E_PAD, 10), np.float32)    # 0..8 sh, 9 valid
        Sd = np.zeros((E_PAD, WIN), BF)    # host one-hot scatter rows
        for w in range(N_WIN):
            nlo = lo + w * WIN
            nhi = min(lo + (w + 1) * WIN, lo + NODES_PER_CORE)
            a = np.searchsorted(dst_s, nlo); b = np.searchsorted(dst_s, nhi)
            idx = order[a:b]
            n = b - a
            assert n <= ESLOT, f"window overflow {n}"
            s = w * ESLOT
            rinT[:8, s:s + n] = rad[idx].T.astype(BF)
            rinT[8:, s:s + n] = attr[idx].T.astype(BF)
            nfsT[:, s:s + n] = nf[src[idx]].T.astype(BF)
            shv[s:s + n, :9] = sh[idx]
            shv[s:s + n, 9] = 1.0
            Sd[np.arange(s, s + n), (dst[idx] - nlo)] = 1.0
        nfT = np.zeros((MUL, N_WIN * WIN), BF)
        nfT[:, :NODES_PER_CORE] = nf[lo:lo + NODES_PER_CORE].T.astype(BF)
        cores.append(dict(rinT=rinT, nfsT=nfsT, shv=shv, Sd=Sd, nfT=nfT))
    return cores


def ref_from_kap(x_kap):
    out = np.empty_like(x_kap)
    for l, d in enumerate(DIMS):
        blk = x_kap[:, LOFF[l]:LOFF[l] + 32 * d].reshape(-1, d, 32)
        out[:, LOFF[l]:LOFF[l] + 32 * d] = np.transpose(blk, (0, 2, 1)).reshape(-1, 32 * d)
    return out


# ---------------------------------------------------------------------------
# device kernel
# ---------------------------------------------------------------------------

_NC_CACHE = {}
LAST_RESULT = None


def build_nc(plan):
    import concourse.bass as bass
    import concourse.bacc as bacc
    import concourse.mybir as mybir
    import concourse.tile as tile

    f32 = mybir.dt.float32
    bf16 = mybir.dt.bfloat16
    AL = mybir.AluOpType
    AF = mybir.ActivationFunctionType

    nc = bacc.Bacc(None)
    P = 128

    # ---- dram parameters
    rinT_d = nc.declare_dram_parameter("rinT", [24, E_PAD], bf16, isOutput=False)
    nfsT_d = nc.declare_dram_parameter("nfsT", [32, E_PAD], bf16, isOutput=False)
    shv_d = nc.declare_dram_parameter("shv", [E_PAD, 10], f32, isOutput=False)
    sd_d = nc.declare_dram_parameter("sd", [E_PAD, WIN], bf16, isOutput=False)
    nfT_d = nc.declare_dram_parameter("nfT", [32, N_WIN * WIN], bf16, isOutput=False)
    omega_d = nc.declare_dram_parameter("omega", [P, plan.totc], bf16, isOutput=False)
    a2_d = nc.declare_dram_parameter("a2", [P, 16 * 96], bf16, isOutput=False)
    b2_d = nc.declare_dram_parameter("b2", [32, 96], bf16, isOutput=False)
    omc1_d = nc.declare_dram_parameter("omc1", [P, HID], bf16, isOutput=False)
    omself_d = nc.declare_dram_parameter("omself", [32, 32], bf16, isOutput=False)
    rw1_d = nc.declare_dram_parameter("rw1", [24, 64], bf16, isOutput=False)
    rw2_d = nc.declare_dram_parameter("rw2", [64, 64], bf16, isOutput=False)
    rw3t2_d = nc.declare_dram_parameter("rw3t2", [64, P], bf16, isOutput=False)
    embrep_d = nc.declare_dram_parameter("embrep", [32, 16 * P], bf16, isOutput=False)
    embb_d = nc.declare_dram_parameter("embb", [32, 32], bf16, isOutput=False)
    rb1_d = nc.declare_dram_parameter("rb1", [64, 1], f32, isOutput=False)
    rb2_d = nc.declare_dram_parameter("rb2", [64, 1], f32, isOutput=False)
    eu8_d = nc.declare_dram_parameter("eu8", [P, 8 * P], bf16, isOutput=False)
    t4q_d = nc.declare_dram_parameter("t4q", [P, P], bf16, isOutput=False)
    identb_d = nc.declare_dram_parameter("identb", [P, P], bf16, isOutput=False)
    zer_d = nc.declare_dram_parameter("zer", [1, P], bf16, isOutput=False)
    zer2_d = nc.declare_dram_parameter("zer2", [1, 1024], bf16, isOutput=False)
    out_d = nc.declare_dram_parameter("out", [N_WIN * WIN, HID], f32, isOutput=True)

    from contextlib import ExitStack
    with tile.TileContext(nc) as tc, ExitStack() as es:
        cst = es.enter_context(tc.tile_pool(name="cst", bufs=1))
        sbw = es.enter_context(tc.tile_pool(name="sbw", bufs=2))
        sb3 = es.enter_context(tc.tile_pool(name="sb3", bufs=3))
        sbf = es.enter_context(tc.tile_pool(name="sbf", bufs=2))
        smT = es.enter_context(tc.tile_pool(name="smT", bufs=2))
        srep = es.enter_context(tc.tile_pool(name="srep", bufs=2))
        stv = es.enter_context(tc.tile_pool(name="stv", bufs=2))
        spt = es.enter_context(tc.tile_pool(name="spt", bufs=3))
        # psum pools (8 banks total)
        pwps = es.enter_context(tc.tile_pool(name="pwps", bufs=1, space="PSUM"))
        ppe1 = es.enter_context(tc.tile_pool(name="ppe1", bufs=1, space="PSUM"))
        pshared = es.enter_context(tc.tile_pool(name="pshared", bufs=2, space="PSUM"))
        prep = es.enter_context(tc.tile_pool(name="prep", bufs=1, space="PSUM"))
        pqm = es.enter_context(tc.tile_pool(name="pqm", bufs=1, space="PSUM"))

        # ---- constants into SBUF
        omega = cst.tile([P, plan.totc], bf16)
        nc.sync.dma_start(out=omega[:], in_=omega_d[:])
        a2 = cst.tile([P, 16 * 96], bf16)
        nc.sync.dma_start(out=a2[:], in_=a2_d[:])
        b2 = cst.tile([32, 96], bf16)
        nc.sync.dma_start(out=b2[:], in_=b2_d[:])
        omc1 = cst.tile([P, HID], bf16)
        nc.sync.dma_start(out=omc1[:], in_=omc1_d[:])
        omself = cst.tile([32, 32], bf16)
        nc.sync.dma_start(out=omself[:], in_=omself_d[:])
        rw1 = cst.tile([24, 64], bf16); nc.sync.dma_start(out=rw1[:], in_=rw1_d[:])
        rw2 = cst.tile([64, 64], bf16); nc.sync.dma_start(out=rw2[:], in_=rw2_d[:])
        rw3t2 = cst.tile([64, P], bf16); nc.sync.dma_start(out=rw3t2[:], in_=rw3t2_d[:])
        embrep = cst.tile([32, 16 * P], bf16); nc.sync.dma_start(out=embrep[:], in_=embrep_d[:])
        embb = cst.tile([32, 32], bf16); nc.sync.dma_start(out=embb[:], in_=embb_d[:])
        rb1 = cst.tile([64, 1], f32); nc.sync.dma_start(out=rb1[:], in_=rb1_d[:])
        rb2 = cst.tile([64, 1], f32); nc.sync.dma_start(out=rb2[:], in_=rb2_d[:])
        eu8 = cst.tile([P, 8 * P], bf16); nc.sync.dma_start(out=eu8[:], in_=eu8_d[:])
        t4q = cst.tile([P, P], bf16); nc.sync.dma_start(out=t4q[:], in_=t4q_d[:])
        identb = cst.tile([P, P], bf16); nc.sync.dma_start(out=identb[:], in_=identb_d[:])
        zer = cst.tile([1, P], bf16); nc.sync.dma_start(out=zer[:], in_=zer_d[:])
        zer2 = cst.tile([1, 1024], bf16); nc.sync.dma_start(out=zer2[:], in_=zer2_d[:])
        nfT = cst.tile([32, N_WIN * WIN], bf16)
        nc.sync.dma_start(out=nfT[:], in_=nfT_d[:])

        def edge_tile(w, j, rin_w, nfs_w, wps):
            t = w * ET_PER_WIN + j
            e0 = t * P
            sh_t = sb3.tile([P, 10], f32, tag="sh")
            nc.sync.dma_start(out=sh_t[:], in_=shv_d[e0:e0 + P, :])
            S_t = sb3.tile([P, P], bf16, tag="S")
            nc.sync.dma_start(out=S_t[:], in_=sd_d[e0:e0 + P, :])

            rin_t = rin_w[:, j * P:(j + 1) * P]
            nfs_t = nfs_w[:, j * P:(j + 1) * P]
            pe1 = ppe1.tile([P, 512], f32, tag="pe1")
            # radial MLP (feature-major); cols: l1 [0:128], l2 [128:256],
            # tile2 [256:384], mxp [384:480]
            nc.tensor.matmul(out=pe1[0:64, 0:128], lhsT=rw1[:], rhs=rin_t, start=True, stop=True)
            f1 = sbf.tile([64, P], bf16, tag="f1")
            nc.scalar.activation(out=f1[:], in_=pe1[0:64, 0:128], func=AF.Silu, bias=rb1[:], scale=1.0)
            nc.tensor.matmul(out=pe1[0:64, 128:256], lhsT=rw2[:], rhs=f1[:], start=True, stop=True)
            f2 = sbf.tile([64, P], bf16, tag="f2")
            nc.scalar.activation(out=f2[:], in_=pe1[0:64, 128:256], func=AF.Silu, bias=rb2[:], scale=1.0)
            # tile2[p, e] = rfT[p%64, e]  (rw3 cols tiled 2x)
            nc.tensor.matmul(out=pe1[:, 256:384], lhsT=rw3t2[:], rhs=f2[:], start=True, stop=True)
            t2b = sbf.tile([P, P], bf16, tag="t2b")
            nc.scalar.copy(out=t2b[:], in_=pe1[:, 256:384])
            # hT (feature-major embedded source scalars)
            hTp = pshared.tile([32, P], f32, tag="scratch")
            nc.tensor.matmul(out=hTp[:], lhsT=embb[:], rhs=nfs_t, start=True, stop=True)
            hTb = sbf.tile([32, P], bf16, tag="hTb")
            nc.scalar.copy(out=hTb[:], in_=hTp[:])
            # monomials mT[(u,r), e] = h[2c+p//64, e] * rfT[p%64, e], 16 chunks
            mT = smT.tile([P, 2048], bf16, tag="mT")
            for qx in range(4):
                mq = pshared.tile([P, 512], f32, tag="scratch")
                for cc in range(4):
                    c = qx * 4 + cc
                    nc.tensor.matmul(out=mq[:, cc * P:(cc + 1) * P],
                                     lhsT=embrep[:, c * P:(c + 1) * P],
                                     rhs=nfs_t, start=True, stop=True)
                mqb = sbf.tile([P, 512], bf16, tag="mqb", bufs=3)
                nc.scalar.copy(out=mqb[:], in_=mq[:])
                enge = nc.gpsimd if qx == 3 else nc.vector
                enge.tensor_tensor(
                    out=mT[:, qx * 512:(qx + 1) * 512].rearrange("p (c z) -> p c z", z=P),
                    in0=mqb[:].rearrange("p (c z) -> p c z", z=P),
                    in1=t2b[:][:, None, :].broadcast_to([P, 4, P]),
                    op=AL.mult)
            # mixed = mT @ A2 + hT @ B2  (PSUM accumulate), mxp = pe1[:,384:480]
            mxp = pe1[:, 384:480]
            for c in range(16):
                nc.tensor.matmul(out=mxp, lhsT=mT[:, c * P:(c + 1) * P],
                                 rhs=a2[:, c * 96:(c + 1) * 96],
                                 start=(c == 0), stop=False)
            nc.tensor.matmul(out=mxp, lhsT=hTb[:], rhs=b2[:], start=False, stop=True)
            # messages: msgs[e, LOFF+i*32+u] = sh[e, SOFF+i] * mixed[e, l*32+u]
            msgs = sb3.tile([P, HID + 1], bf16, tag="msgs")
            for l, d in enumerate(DIMS):
                nc.vector.tensor_tensor(
                    out=msgs[:, LOFF[l]:LOFF[l] + 32 * d].rearrange("p (i u) -> p i u", u=32),
                    in0=sh_t[:, SOFF[l]:SOFF[l] + d][:, :, None].broadcast_to([P, d, 32]),
                    in1=mxp[:, l * 32:(l + 1) * 32][:, None, :].broadcast_to([P, d, 32]),
                    op=AL.mult)
            nc.vector.tensor_copy(out=msgs[:, HID:HID + 1], in_=sh_t[:, 9:10])
            # scatter-accumulate into window psum
            nc.tensor.matmul(out=wps[:], lhsT=S_t[:], rhs=msgs[:],
                             start=(j == 0), stop=(j == ET_PER_WIN - 1))

        # J-columns present (all 9 irrep rows)
        JLIST = [(l, i) for l in range(3) for i in range(DIMS[l])]
        JIDX = {Ij: n for n, Ij in enumerate(JLIST)}

        def build_rep(xT2, I, blkcnt):
            """Replicated tile for irrep-row I from row-block-major xT2
            ([32, 9*128], block jn holds features cJ..cJ+32 on partitions
            0:32). Returns SBUF bf16 [128, 1024]: chunk kc col-block holds
            xT2-row (4kc + p//32) of block I at partition p."""
            jn = JIDX[I]
            pr = prep.tile([P, 1024], f32, tag="rep")
            for kc in range(8):
                nc.tensor.matmul(out=pr[:, kc * P:(kc + 1) * P],
                                 lhsT=eu8[0:32, kc * P:(kc + 1) * P],
                                 rhs=xT2[0:32, jn * P:(jn + 1) * P],
                                 start=True, stop=True)
            rep_sb = srep.tile([P, 1024], bf16, tag="repsb")
            nc.scalar.copy(out=rep_sb[:], in_=pr[:])
            return rep_sb

        def make_xT2(x_bf, tagp):
            """row-block transpose of x_bf [128, 288] -> [32, 9*128] bf16:
            block jn holds x_bf[:, cJ:cJ+32]^T on partitions 0:32."""
            tpa = pshared.tile([32, 5 * P], bf16, tag="scratch")
            for jn in range(5):
                (l, i) = JLIST[jn]
                cJ = LOFF[l] + i * 32
                nc.tensor.transpose(out=tpa[:, jn * P:(jn + 1) * P],
                                    in_=x_bf[:, cJ:cJ + 32], identity=identb[:])
            tpb = pshared.tile([32, 4 * P], bf16, tag="scratch")
            for jn in range(5, 9):
                (l, i) = JLIST[jn]
                cJ = LOFF[l] + i * 32
                nc.tensor.transpose(out=tpb[:, (jn - 5) * P:(jn - 4) * P],
                                    in_=x_bf[:, cJ:cJ + 32], identity=identb[:])
            xT2 = stv.tile([32, 9 * P], bf16, tag=tagp)
            nc.vector.tensor_copy(out=xT2[:, 0:5 * P], in_=tpa[:])
            nc.vector.tensor_copy(out=xT2[:, 5 * P:9 * P], in_=tpb[:])
            return xT2

        def make_aT(x_bf):
            """feature-major transpose of x_bf [128, 288] -> [128, 384] bf16
            (for the c1 block-diagonal matmuls)."""
            tp = pshared.tile([P, 384], bf16, tag="scratch")
            nc.tensor.transpose(out=tp[:, 0:P], in_=x_bf[:, 0:P], identity=identb[:])
            nc.tensor.transpose(out=tp[:, P:2 * P], in_=x_bf[:, P:2 * P], identity=identb[:])
            nc.tensor.transpose(out=tp[0:32, 2 * P:3 * P], in_=x_bf[:, 2 * P:HID], identity=identb[:])
            aT = stv.tile([P, 3 * P], bf16, tag="aT")
            nc.vector.tensor_copy(out=aT[:, 0:2 * P], in_=tp[:, 0:2 * P])
            nc.vector.tensor_copy(out=aT[0:32, 2 * P:3 * P], in_=tp[0:32, 2 * P:3 * P])
            return aT

        blk_counter = [0]

        def do_pass(blocks, emi_list, xT2, tile_v, qm_ps):
            # group blocks by I (list is sorted by (I, J))
            bi = 0
            nb = len(blocks)
            while bi < nb:
                I = blocks[bi][0]
                bj = bi
                while bj < nb and blocks[bj][0] == I:
                    bj += 1
                rep_sb = build_rep(xT2, I, bj - bi)
                for bk in range(bi, bj):
                    J = blocks[bk][1]
                    jn = JIDX[J]
                    PT = spt.tile([P, 1024], bf16, tag="PT")
                    eng = nc.gpsimd if blk_counter[0] % 16 in (3, 7, 11, 13, 15) else nc.vector
                    blk_counter[0] += 1
                    eng.tensor_tensor(
                        out=PT[:].rearrange("p (c z) -> p c z", z=P),
                        in0=rep_sb[:].rearrange("p (c z) -> p c z", z=P),
                        in1=tile_v[:, jn * P:(jn + 1) * P][:, None, :].broadcast_to([P, 8, P]),
                        op=AL.mult)
                    for kc in range(8):
                        for (c0, c1, off) in emi_list[bk][kc]:
                            nc.tensor.matmul(out=qm_ps[:, c0:c1],
                                             lhsT=PT[:, kc * P:(kc + 1) * P],
                                             rhs=omega[:, off:off + (c1 - c0)],
                                             start=False, stop=False,
                                             skip_group_check=True)
                bi = bj

        def node_window(w, wps):
            # normalize: a = wps[:, :288] / max(cnt, 1)
            cnt = sbf.tile([P, 1], f32, tag="cnt")
            nc.vector.tensor_scalar_max(out=cnt[:], in0=wps[:, HID:HID + 1], scalar1=1.0)
            rec = sbf.tile([P, 1], f32, tag="rec")
            nc.vector.reciprocal(out=rec[:], in_=cnt[:])
            a_bf = stv.tile([P, HID], bf16, tag="abf")
            nc.vector.tensor_scalar_mul(out=a_bf[:], in0=wps[:, :HID], scalar1=rec[:])
            aT = make_aT(a_bf)
            aT2 = make_xT2(a_bf, "aT2")
            # tile_v[:, jn*128:(jn+1)*128][p, z] = a[z, cJ + p%32]
            tile_v = stv.tile([P, 9 * P], bf16, tag="tv")
            for g in range(2):
                jlo = g * 5
                jhi = min(jlo + 5, 9)
                pv = prep.tile([P, 1024], f32, tag="rep")
                for jj in range(jlo, jhi):
                    nc.tensor.matmul(out=pv[:, (jj - jlo) * P:(jj - jlo + 1) * P],
                                     lhsT=t4q[0:32, :],
                                     rhs=aT2[0:32, jj * P:(jj + 1) * P],
                                     start=True, stop=True)
                nc.scalar.copy(out=tile_v[:, jlo * P:jhi * P],
                               in_=pv[:, 0:(jhi - jlo) * P])

            # qm layout: q at cols [0:288) (bank 0), msg at [512:800) (bank 1)
            MB = 512
            qm_ps = pqm.tile([P, 1024], f32, tag="qm")
            nc.tensor.matmul(out=qm_ps[:, 0:512], lhsT=zer[:, 0:P], rhs=zer2[:, 0:512], start=True, stop=False)
            nc.tensor.matmul(out=qm_ps[:, 512:1024], lhsT=zer[:, 0:P], rhs=zer2[:, 512:1024], start=True, stop=False)

            do_pass(plan.aa, plan.aa_emi, aT2, tile_v, qm_ps)
            # close bank-0 accumulation group, then evacuate q to bf16
            nc.tensor.matmul(out=qm_ps[:, 0:HID], lhsT=zer[:, 0:P], rhs=zer2[:, 0:HID],
                             start=False, stop=True)
            q_bf = stv.tile([P, HID], bf16, tag="qbf")
            nc.scalar.copy(out=q_bf[:], in_=qm_ps[:, 0:HID])
            qT2 = make_xT2(q_bf, "qT2")
            do_pass(plan.qa, plan.qa_emi, qT2, tile_v, qm_ps)
            # c1: msg += aT-chunks @ omc1-chunks  (block-diagonal linear)
            nc.tensor.matmul(out=qm_ps[:, MB + 0:MB + 128], lhsT=aT[:, 0:P],
                             rhs=omc1[:, 0:128], start=False, stop=False,
                             skip_group_check=True)
            nc.tensor.matmul(out=qm_ps[:, MB + 128:MB + 256], lhsT=aT[:, P:2 * P],
                             rhs=omc1[:, 128:256], start=False, stop=False,
                             skip_group_check=True)
            nc.tensor.matmul(out=qm_ps[:, MB + 256:MB + 288], lhsT=aT[0:32, 2 * P:3 * P],
                             rhs=omc1[0:32, 256:288], start=False,
                             stop=False, skip_group_check=True)
            # self connection (l=0 cols)
            nc.tensor.matmul(out=qm_ps[:, MB:MB + 32], lhsT=nfT[:, w * P:(w + 1) * P],
                             rhs=omself[:], start=False, stop=False,
                             skip_group_check=True)
            # close bank-1 group
            nc.tensor.matmul(out=qm_ps[:, MB:MB + HID], lhsT=zer[:, 0:P], rhs=zer2[:, 0:HID],
                             start=False, stop=True)
            # write out (DMA cannot read PSUM; bounce via SBUF)
            out_sb = stv.tile([P, HID], f32, tag="outsb")
            nc.scalar.copy(out=out_sb[:], in_=qm_ps[:, MB:MB + HID])
            nc.sync.dma_start(out=out_d[w * P:(w + 1) * P, :], in_=out_sb[:])

        # ---------------- main loop ----------------
        for w in range(N_WIN):
            rin_w = sbw.tile([24, ESLOT], bf16, tag="rinw")
            nc.sync.dma_start(out=rin_w[:], in_=rinT_d[:, w * ESLOT:(w + 1) * ESLOT])
            nfs_w = sbw.tile([32, ESLOT], bf16, tag="nfsw")
            nc.sync.dma_start(out=nfs_w[:], in_=nfsT_d[:, w * ESLOT:(w + 1) * ESLOT])
            wps = pwps.tile([P, HID + 1], f32, tag="wps")
            for j in range(ET_PER_WIN):
                edge_tile(w, j, rin_w, nfs_w, wps)
            node_window(w, wps)

    nc.finalize()
    return nc


def _get_nc(plan):
    if 'nc' not in _NC_CACHE:
        _NC_CACHE['nc'] = build_nc(plan)
    return _NC_CACHE['nc']


def kernel(**inputs):
    global LAST_RESULT
    from concourse.bass_utils import run_bass_kernel_spmd

    inp = {k: np.asarray(v) for k, v in inputs.items()}
    plan = build_plan()
    W = fold_weights(inp)
    om = pack_omega(plan, W)

    # A2 repacked for (u-major-2, r-minor-64) monomial chunks:
    # chunk c, row p  ->  A2 row (p%64)*32 + (2c + p//64)
    A2 = W['A2'].astype(np.float32)
    a2p = np.zeros((128, 16 * 96), np.float32)
    pidx = np.arange(128)
    for c in range(16):
        rows = (pidx % 64) * 32 + (2 * c + pidx // 64)
        a2p[:, c * 96:(c + 1) * 96] = A2[rows, :]
    # fold b3 into B2: mixed gets (rf0 + b3) x h terms; b3 (x) h part is linear in h
    B2 = W['B2'].astype(np.float64).copy()
    b3 = inp['r_b3'].astype(np.float64)
    for u in range(32):
        B2[u, :] += b3 @ A2[np.arange(RHID) * 32 + u, :].astype(np.float64)
    # omc1 packed [128, 288]
    omc1 = W['omc1']
    omc1p = np.zeros((128, HID), np.float32)
    omc1p[:, 0:128] = omc1[0:128, 0:128]
    omc1p[:, 128:256] = omc1[128:256, 128:256]
    omc1p[0:32, 256:288] = omc1[256:288, 256:288]

    identb = np.eye(128, dtype=np.float32).astype(BF)
    # eu8: quadrant-replicated row-replication selectors
    eu8 = np.zeros((128, 8 * 128), np.float32)
    for q in range(4):
        for kc in range(8):
            for p in range(128):
                eu8[32 * q + (4 * kc + p // 32) % 32, kc * 128 + p] = 1.0
    # t4q: quadrant-replicated 4x row tiling
    t4q = np.zeros((128, 128), np.float32)
    for q in range(4):
        for p in range(128):
            t4q[32 * q + p % 32, p] = 1.0
    # rw3 cols tiled 2x: tile2[p, e] = rfT[p%64, e]
    rw3 = W['r_w3'].astype(np.float64)
    rw3t2 = np.zeros((64, 128), np.float64)
    rw3t2[:, 0:64] = rw3
    rw3t2[:, 64:128] = rw3
    # emb columns replicated: embrep chunk c col p -> emb col (2c + p//64)
    emb = W['emb']
    embrep = np.zeros((32, 16 * 128), np.float64)
    for c in range(16):
        for p in range(128):
            embrep[:, c * 128 + p] = emb[:, 2 * c + p // 64]

    shared = dict(
        omega=om,
        a2=a2p.astype(BF), b2=B2.astype(np.float32).astype(BF),
        omc1=omc1p.astype(BF), omself=W['omself'].astype(np.float32).astype(BF),
        rw1=W['r_w1'].astype(BF), rw2=W['r_w2'].astype(BF),
        rw3t2=rw3t2.astype(np.float32).astype(BF),
        embrep=embrep.astype(np.float32).astype(BF),
        embb=emb.astype(np.float32).astype(BF),
        rb1=W['r_b1'].reshape(64, 1), rb2=W['r_b2'].reshape(64, 1),
        eu8=eu8.astype(BF), t4q=t4q.astype(BF), identb=identb,
        zer=np.zeros((1, 128), BF),
        zer2=np.zeros((1, 1024), BF),
    )
    cores = pack_edges(inp)
    in_maps = []
    for c in range(N_CORES):
        m = dict(shared)
        m.update(rinT=cores[c]['rinT'], nfsT=cores[c]['nfsT'],
                 shv=cores[c]['shv'], sd=cores[c]['Sd'], nfT=cores[c]['nfT'])
        in_maps.append(m)

    nc = _get_nc(plan)
    res = run_bass_kernel_spmd(nc, in_maps, core_ids=list(range(N_CORES)))
    LAST_RESULT = res
    outs = [res.results[c]['out'][:NODES_PER_CORE] for c in range(N_CORES)]
    out_kap = np.concatenate(outs, axis=0).astype(np.float32)
    return ref_from_kap(out_kap)


if __name__ == "__main__":
    plan = build_plan()
    print(f"aa blocks: {len(plan.aa)}  qa blocks: {len(plan.qa)}")
    print(f"omega cols: {plan.totc}  ({plan.totc * 128 * 2 / 1e6:.1f} MB bf16)")
    print(f"emissions per node-tile: {plan.n_emi}")
    if len(sys.argv) > 1 and sys.argv[1] == '--build':
        nc = build_nc(plan)
        print("build OK")


# revision 38
# speedup vs baseline: 1.0822x; 1.0822x over previous
"""Trainium2 Bass kernel for nn_MACEConvolutionLayer.

Strategy (8 NeuronCores, no collectives):
  - Edges are sharded by DESTINATION node range (1250 nodes/core), sorted and
    packed into 10 windows of 128 nodes x 1024 edge slots per core. Each core
    computes messages for its edge shard and segment-sums them into its own
    node shard via host-precomputed one-hot matmuls on the tensor engine.
  - Per-edge bilinear (radial-MLP features x embedded source scalars) and the
    per-node equivariant tensor products use a monomial scheme in TRANSPOSED
    (feature-major) layout: PE "replication matmuls" (constant selection
    matrices as lhsT) build row-replicated/tiled operand tiles in PSUM, the
    scalar engine casts them to bf16 SBUF, and the DVE forms the monomial
    products with stride-1 access patterns (2x mode). PE then contracts the
    monomial chunks against packed omega matrices (Clebsch-Gordan x TP
    weights, with channel-mixing/combination linears folded in).

Feature layout on device is kappa-major: col(l, i, u) = LOFF[l] + i*32 + u.
"""
import sys, os

sys.path.insert(0, '/opt/trn_rl_repo')

import numpy as np
import ml_dtypes

MUL = 32
DIMS = (1, 3, 5)
HID = 288
N_NODES = 10000
N_EDGES = 64000
RHID = 64
SQM = float(np.sqrt(MUL))
LOFF = [0, 32, 128]
SOFF = [0, 1, 4]
PATHS_FULL = [(0,0,0),(0,1,1),(0,2,2),(1,0,1),(1,1,0),(1,1,2),(1,2,1),(2,0,2),(2,1,1),(2,2,0),(2,2,2)]
O2_UVW = [(0,1,1),(0,2,2),(1,2,1)]
O2_UVU = [(0,0,0),(1,1,0),(1,1,2),(2,2,0),(2,2,2)]

N_CORES = 8
NODES_PER_CORE = 1250
WIN = 128
N_WIN = 10
ESLOT = 896
E_PAD = N_WIN * ESLOT   # 8960
ET_PER_WIN = ESLOT // 128  # 7
BF = ml_dtypes.bfloat16


def cg_np():
    s2, s3, s5, s6 = map(np.sqrt, (2.0, 3.0, 5.0, 6.0))
    B = np.zeros((5, 3, 3))
    B[0, 0, 1] = B[0, 1, 0] = 1 / s2
    B[1, 1, 2] = B[1, 2, 1] = 1 / s2
    B[2] = np.diag([-1.0, -1.0, 2.0]) / s6
    B[3, 0, 2] = B[3, 2, 0] = 1 / s2
    B[4] = np.diag([1.0, -1.0, 0.0]) / s2
    C = {}
    C[(0, 0, 0)] = np.ones((1, 1, 1))
    C[(0, 1, 1)] = (np.eye(3) / s3)[None]
    C[(1, 0, 1)] = np.transpose(C[(0, 1, 1)], (1, 0, 2))
    C[(0, 2, 2)] = (np.eye(5) / s5)[None]
    C[(2, 0, 2)] = np.transpose(C[(0, 2, 2)], (1, 0, 2))
    C[(1, 1, 0)] = (np.eye(3) / s3)[:, :, None]
    C[(1, 1, 2)] = np.transpose(B, (1, 2, 0)) / s5
    C[(1, 2, 1)] = np.transpose(B, (1, 0, 2)) / s5
    C[(2, 1, 1)] = B / s5
    C[(2, 2, 0)] = (np.eye(5) / s5)[:, :, None]
    T = np.einsum('aij,bjk,cki->abc', B, B, B)
    C[(2, 2, 2)] = T / np.linalg.norm(T)
    return C

CG = cg_np()
PATH_LIST_O2 = O2_UVW + O2_UVU  # (i,j,k) in folded order


def support_pairs(path_ijk):
    d = {}
    for pi, (li, lj, lk) in enumerate(path_ijk):
        C = CG[(li, lj, lk)]
        for iloc in range(DIMS[li]):
            for jloc in range(DIMS[lj]):
                if np.any(np.abs(C[iloc, jloc, :]) > 1e-12):
                    d.setdefault(((li, iloc), (lj, jloc)), []).append((pi, iloc, jloc))
    return d


def build_mono_blocks_sym(path_ijk):
    d = support_pairs(path_ijk)
    blocks = {}
    for (I, J), lst in d.items():
        key = (min(I, J), max(I, J))
        swap = I > J
        for (pi, iloc, jloc) in lst:
            blocks.setdefault(key, []).append((pi, iloc, jloc, swap))
    return [(I, J, c) for (I, J), c in sorted(blocks.items())]


def build_mono_blocks(path_ijk):
    d = support_pairs(path_ijk)
    return [(I, J, [(pi, i, j, False) for (pi, i, j) in lst]) for (I, J), lst in sorted(d.items())]


def omega_for_block(path_ijk, weights, I, J, contribs):
    """[1024 (u-major,v-fast), 288] kappa-major outputs."""
    Om = np.zeros((MUL * MUL, HID))
    for (pi, iloc, jloc, swap) in contribs:
        li, lj, lk = path_ijk[pi]
        W = weights[pi]
        C = CG[(li, lj, lk)]
        for kap in range(DIMS[lk]):
            c = C[iloc, jloc, kap]
            if abs(c) < 1e-12:
                continue
            c0 = LOFF[lk] + kap * 32
            Wm = W if not swap else np.transpose(W, (1, 0, 2))
            Om[:, c0:c0 + 32] += c * Wm.reshape(MUL * MUL, MUL)
    return Om


# ---------------------------------------------------------------------------
# static plan: monomial blocks + emission structure (depends only on CG)
# ---------------------------------------------------------------------------

class Plan:
    pass


GAPTOL = 4

def _emissions_for_pair(mq, mm_):
    """Emissions in the device qm space: q outputs at cols [0:288) (PSUM
    bank 0), msg outputs at cols [512:800) (bank 1). mq/mm_: [1024, 288]
    bool masks (or None). Returns list over kchunk of list of (c0, c1).
    Runs never merge across the q/msg boundary."""
    out = []
    for kc in range(8):
        emis = []
        for mask, coff in ((mq, 0), (mm_, 512)):
            if mask is None:
                continue
            sub = mask[kc * 128:(kc + 1) * 128]
            used = [g for g in range(9) if np.any(sub[:, g * 32:(g + 1) * 32])]
            if not used:
                continue
            runs = [[used[0], used[0] + 1]]
            for g in used[1:]:
                if g - runs[-1][1] <= GAPTOL:
                    runs[-1][1] = g + 1
                else:
                    runs.append([g, g + 1])
            for (ga, gb) in runs:
                emis.append((coff + ga * 32, coff + gb * 32))
        out.append(emis)
    return out


def build_plan():
    p = Plan()
    p.aa_blocks = build_mono_blocks_sym(PATHS_FULL + PATH_LIST_O2)
    p.qa_blocks = build_mono_blocks(PATHS_FULL)
    n3a = len(PATHS_FULL)
    ones_a = [np.ones((MUL, MUL, MUL)) for _ in PATHS_FULL]
    ones_o2 = [np.ones((MUL, MUL, MUL)) for _ in PATH_LIST_O2]

    # per aa block: contribs split into q-part (o3a) and msg-part (o2)
    p.aa = []
    for (I, J, contribs) in p.aa_blocks:
        cq = [(pi, i, j, s) for (pi, i, j, s) in contribs if pi < n3a]
        cm = [(pi - n3a, i, j, s) for (pi, i, j, s) in contribs if pi >= n3a]
        mq = omega_for_block(PATHS_FULL, ones_a, I, J, cq) != 0 if cq else None
        mm_ = omega_for_block(PATH_LIST_O2, ones_o2, I, J, cm) != 0 if cm else None
        em = _emissions_for_pair(mq, mm_)
        p.aa.append((I, J, cq, cm, em))
    p.qa = []
    for (I, J, contribs) in p.qa_blocks:
        mm_ = omega_for_block(PATHS_FULL, ones_a, I, J, contribs) != 0
        em = _emissions_for_pair(None, mm_)
        p.qa.append((I, J, contribs, em))

    # assign omega column offsets; emission = (c0, c1, om_off) combined space
    off = 0
    p.aa_emi = []
    for (I, J, cq, cm, em) in p.aa:
        bk = []
        for kc in range(8):
            lst = []
            for (c0, c1) in em[kc]:
                lst.append((c0, c1, off))
                off += c1 - c0
            bk.append(lst)
        p.aa_emi.append(bk)
    p.qa_emi = []
    for (I, J, contribs, em) in p.qa:
        bk = []
        for kc in range(8):
            lst = []
            for (c0, c1) in em[kc]:
                lst.append((c0, c1, off))
                off += c1 - c0
            bk.append(lst)
        p.qa_emi.append(bk)
    p.totc = off
    n_emi = sum(len(l) for bk in p.aa_emi + p.qa_emi for l in bk)
    p.n_emi = n_emi
    return p


def pack_omega(plan, Wfold):
    """Fill the packed omega array [128, totc] (bf16) from folded weights.
    Emission col-ranges live in the combined 576-col space (q | msg)."""
    W3a = Wfold['o3a_w']; Wo2 = Wfold['o2_w']; W3b = Wfold['o3b_w']
    om = np.zeros((128, plan.totc), np.float32)

    def omcol(c):
        # device qm col -> Om col (q at [0:288), msg at [512:800) -> [288:576))
        return c - 224 if c >= 512 else c

    for bi, (I, J, cq, cm, em) in enumerate(plan.aa):
        Om = np.zeros((MUL * MUL, 2 * HID))
        if cq:
            Om[:, :HID] = omega_for_block(PATHS_FULL, W3a, I, J, cq)
        if cm:
            Om[:, HID:] = omega_for_block(PATH_LIST_O2, Wo2, I, J, cm)
        for kc in range(8):
            for (c0, c1, off) in plan.aa_emi[bi][kc]:
                om[:, off:off + (c1 - c0)] = Om[kc * 128:(kc + 1) * 128, omcol(c0):omcol(c0) + (c1 - c0)]
    for bi, (I, J, contribs, em) in enumerate(plan.qa):
        Om = np.zeros((MUL * MUL, 2 * HID))
        Om[:, HID:] = omega_for_block(PATHS_FULL, W3b, I, J, contribs)
        for kc in range(8):
            for (c0, c1, off) in plan.qa_emi[bi][kc]:
                om[:, off:off + (c1 - c0)] = Om[kc * 128:(kc + 1) * 128, omcol(c0):omcol(c0) + (c1 - c0)]
    return om.astype(BF)


def fold_weights(inp):
    f8 = np.float64
    mix_w = inp['mix_w'].astype(f8); comb_w = inp['comb_w'].astype(f8)
    M = np.einsum('olux,olxw->oluw', mix_w, comb_w) / MUL
    W1eff = np.einsum('lux,lxw->luw', inp['lin_o1'].astype(f8), M[0]) / SQM
    o2_w = []
    for pp, (i, j, k) in enumerate(O2_UVW):
        o2_w.append(np.einsum('uvx,xw->uvw', inp['o2_uvw'][pp].astype(f8) / MUL, M[1][k]))
    for pp, (i, j, k) in enumerate(O2_UVU):
        o2_w.append(np.einsum('uv,uw->uvw', inp['o2_uvu'][pp].astype(f8), M[1][k]) / SQM)
    o3a_w = [inp['o3a_uvw'][pp].astype(f8) / MUL for pp in range(len(PATHS_FULL))]
    o3b_w = [np.einsum('uvx,xw->uvw', inp['o3b_uvw'][pp].astype(f8) / MUL, M[2][k])
             for pp, (i, j, k) in enumerate(PATHS_FULL)]
    aw = inp['a_w'].astype(f8).reshape(RHID, 3, MUL, MUL)
    ab = inp['a_b'].astype(f8).reshape(3, MUL, MUL)
    scale = np.array([1.0 / np.sqrt(d) for d in DIMS]) / SQM
    aw = aw * scale[None, :, None, None]
    ab = ab * scale[:, None, None]
    A2 = np.transpose(aw, (0, 2, 1, 3)).reshape(RHID * MUL, 3 * MUL)
    B2 = np.transpose(ab, (1, 0, 2)).reshape(MUL, 3 * MUL)
    # c1 block-diagonal omegas per aT chunk (kappa-major rows/cols)
    omc1 = np.zeros((HID, HID))
    for l in range(3):
        for i in range(DIMS[l]):
            c = LOFF[l] + i * 32
            omc1[c:c + 32, c:c + 32] = W1eff[l]
    return dict(
        o3a_w=o3a_w, o2_w=o2_w, o3b_w=o3b_w,
        omc1=omc1, omself=inp['self_w'].astype(f8) / SQM,
        emb=inp['emb_w'].astype(f8) / SQM,
        A2=A2, B2=B2,
        r_w1=inp['r_w1'].astype(np.float32), r_b1=inp['r_b1'].astype(np.float32),
        r_w2=inp['r_w2'].astype(np.float32), r_b2=inp['r_b2'].astype(np.float32),
        r_w3=inp['r_w3'].astype(np.float32), r_b3=inp['r_b3'].astype(np.float32),
    )


def pack_edges(inp):
    src = np.asarray(inp['edge_index'][0]).astype(np.int64)
    dst = np.asarray(inp['edge_index'][1]).astype(np.int64)
    sh = np.asarray(inp['edge_sh'], dtype=np.float32)
    rad = np.asarray(inp['edge_radial_embedding'], dtype=np.float32)
    attr = np.asarray(inp['edge_attr'], dtype=np.float32)
    nf = np.asarray(inp['node_features'], dtype=np.float32)
    deg_all = np.bincount(dst, minlength=N_NODES)
    cores = []
    for c in range(N_CORES):
        lo = c * NODES_PER_CORE
        deg = deg_all[lo:lo + NODES_PER_CORE]
        # greedy degree-balanced binning of this core's nodes into N_WIN
        # windows of <= 128 nodes each
        srt = np.argsort(-deg, kind='stable')
        bin_edges = np.zeros(N_WIN, np.int64)
        bin_cnt = np.zeros(N_WIN, np.int64)
        bin_of = np.zeros(NODES_PER_CORE, np.int64)
        slot_of = np.zeros(NODES_PER_CORE, np.int64)
        for nl in srt:
            cand = np.nonzero(bin_cnt < WIN)[0]
            b = cand[np.argmin(bin_edges[cand])]
            bin_of[nl] = b; slot_of[nl] = bin_cnt[b]
            bin_cnt[b] += 1; bin_edges[b] += deg[nl]
        assert bin_edges.max() <= ESLOT, f"window overflow {bin_edges.max()}"
        pos = bin_of * WIN + slot_of      # node nl -> device output row
        rinT = np.zeros((24, E_PAD), BF)
        nfsT = np.zeros((MUL, E_PAD), BF)
        shv = np.zeros((E_PAD, 10), np.float32)    # 0..8 sh, 9 valid
        Sd = np.zeros((E_PAD, WIN), BF)    # host one-hot scatter rows
        nfT = np.zeros((MUL, N_WIN * WIN), BF)
        nfT[:, pos] = nf[lo:lo + NODES_PER_CORE].T.astype(BF)
        esel = np.nonzero((dst >= lo) & (dst < lo + NODES_PER_CORE))[0]
        ebin = bin_of[dst[esel] - lo]
        for w in range(N_WIN):
            idx = esel[ebin == w]
            n = len(idx)
            s = w * ESLOT
            rinT[:8, s:s + n] = rad[idx].T.astype(BF)
            rinT[8:, s:s + n] = attr[idx].T.astype(BF)
            nfsT[:, s:s + n] = nf[src[idx]].T.astype(BF)
            shv[s:s + n, :9] = sh[idx]
            shv[s:s + n, 9] = 1.0
            Sd[np.arange(s, s + n), slot_of[dst[idx] - lo]] = 1.0
        cores.append(dict(rinT=rinT, nfsT=nfsT, shv=shv, Sd=Sd, nfT=nfT, pos=pos))
    return cores


def ref_from_kap(x_kap):
    out = np.empty_like(x_kap)
    for l, d in enumerate(DIMS):
        blk = x_kap[:, LOFF[l]:LOFF[l] + 32 * d].reshape(-1, d, 32)
        out[:, LOFF[l]:LOFF[l] + 32 * d] = np.transpose(blk, (0, 2, 1)).reshape(-1, 32 * d)
    return out


# ---------------------------------------------------------------------------
# device kernel
# ---------------------------------------------------------------------------

_NC_CACHE = {}
LAST_RESULT = None


def build_nc(plan):
    import concourse.bass as bass
    import concourse.bacc as bacc
    import concourse.mybir as mybir
    import concourse.tile as tile

    f32 = mybir.dt.float32
    bf16 = mybir.dt.bfloat16
    AL = mybir.AluOpType
    AF = mybir.ActivationFunctionType

    nc = bacc.Bacc(None)
    P = 128

    # ---- dram parameters
    rinT_d = nc.declare_dram_parameter("rinT", [24, E_PAD], bf16, isOutput=False)
    nfsT_d = nc.declare_dram_parameter("nfsT", [32, E_PAD], bf16, isOutput=False)
    shv_d = nc.declare_dram_parameter("shv", [E_PAD, 10], f32, isOutput=False)
    sd_d = nc.declare_dram_parameter("sd", [E_PAD, WIN], bf16, isOutput=False)
    nfT_d = nc.declare_dram_parameter("nfT", [32, N_WIN * WIN], bf16, isOutput=False)
    omega_d = nc.declare_dram_parameter("omega", [P, plan.totc], bf16, isOutput=False)
    a2_d = nc.declare_dram_parameter("a2", [P, 16 * 96], bf16, isOutput=False)
    b2_d = nc.declare_dram_parameter("b2", [32, 96], bf16, isOutput=False)
    omc1_d = nc.declare_dram_parameter("omc1", [P, HID], bf16, isOutput=False)
    omself_d = nc.declare_dram_parameter("omself", [32, 32], bf16, isOutput=False)
    rw1_d = nc.declare_dram_parameter("rw1", [24, 64], bf16, isOutput=False)
    rw2_d = nc.declare_dram_parameter("rw2", [64, 64], bf16, isOutput=False)
    rw3t2_d = nc.declare_dram_parameter("rw3t2", [64, P], bf16, isOutput=False)
    embrep_d = nc.declare_dram_parameter("embrep", [32, 16 * P], bf16, isOutput=False)
    embb_d = nc.declare_dram_parameter("embb", [32, 32], bf16, isOutput=False)
    rb1_d = nc.declare_dram_parameter("rb1", [64, 1], f32, isOutput=False)
    rb2_d = nc.declare_dram_parameter("rb2", [64, 1], f32, isOutput=False)
    eu8_d = nc.declare_dram_parameter("eu8", [P, 8 * P], bf16, isOutput=False)
    t4q_d = nc.declare_dram_parameter("t4q", [P, P], bf16, isOutput=False)
    identb_d = nc.declare_dram_parameter("identb", [P, P], bf16, isOutput=False)
    zer_d = nc.declare_dram_parameter("zer", [1, P], bf16, isOutput=False)
    zer2_d = nc.declare_dram_parameter("zer2", [1, 1024], bf16, isOutput=False)
    out_d = nc.declare_dram_parameter("out", [N_WIN * WIN, HID], f32, isOutput=True)

    from contextlib import ExitStack
    with tile.TileContext(nc) as tc, ExitStack() as es:
        cst = es.enter_context(tc.tile_pool(name="cst", bufs=1))
        sbw = es.enter_context(tc.tile_pool(name="sbw", bufs=2))
        sb3 = es.enter_context(tc.tile_pool(name="sb3", bufs=3))
        sbf = es.enter_context(tc.tile_pool(name="sbf", bufs=2))
        smT = es.enter_context(tc.tile_pool(name="smT", bufs=2))
        srep = es.enter_context(tc.tile_pool(name="srep", bufs=2))
        stv = es.enter_context(tc.tile_pool(name="stv", bufs=2))
        spt = es.enter_context(tc.tile_pool(name="spt", bufs=3))
        # psum pools (8 banks total)
        pwps = es.enter_context(tc.tile_pool(name="pwps", bufs=1, space="PSUM"))
        ppe1 = es.enter_context(tc.tile_pool(name="ppe1", bufs=1, space="PSUM"))
        pshared = es.enter_context(tc.tile_pool(name="pshared", bufs=2, space="PSUM"))
        prep = es.enter_context(tc.tile_pool(name="prep", bufs=1, space="PSUM"))
        pqm = es.enter_context(tc.tile_pool(name="pqm", bufs=1, space="PSUM"))

        # ---- constants into SBUF
        omega = cst.tile([P, plan.totc], bf16)
        nc.sync.dma_start(out=omega[:], in_=omega_d[:])
        a2 = cst.tile([P, 16 * 96], bf16)
        nc.sync.dma_start(out=a2[:], in_=a2_d[:])
        b2 = cst.tile([32, 96], bf16)
        nc.sync.dma_start(out=b2[:], in_=b2_d[:])
        omc1 = cst.tile([P, HID], bf16)
        nc.sync.dma_start(out=omc1[:], in_=omc1_d[:])
        omself = cst.tile([32, 32], bf16)
        nc.sync.dma_start(out=omself[:], in_=omself_d[:])
        rw1 = cst.tile([24, 64], bf16); nc.sync.dma_start(out=rw1[:], in_=rw1_d[:])
        rw2 = cst.tile([64, 64], bf16); nc.sync.dma_start(out=rw2[:], in_=rw2_d[:])
        rw3t2 = cst.tile([64, P], bf16); nc.sync.dma_start(out=rw3t2[:], in_=rw3t2_d[:])
        embrep = cst.tile([32, 16 * P], bf16); nc.sync.dma_start(out=embrep[:], in_=embrep_d[:])
        embb = cst.tile([32, 32], bf16); nc.sync.dma_start(out=embb[:], in_=embb_d[:])
        rb1 = cst.tile([64, 1], f32); nc.sync.dma_start(out=rb1[:], in_=rb1_d[:])
        rb2 = cst.tile([64, 1], f32); nc.sync.dma_start(out=rb2[:], in_=rb2_d[:])
        eu8 = cst.tile([P, 8 * P], bf16); nc.sync.dma_start(out=eu8[:], in_=eu8_d[:])
        t4q = cst.tile([P, P], bf16); nc.sync.dma_start(out=t4q[:], in_=t4q_d[:])
        identb = cst.tile([P, P], bf16); nc.sync.dma_start(out=identb[:], in_=identb_d[:])
        zer = cst.tile([1, P], bf16); nc.sync.dma_start(out=zer[:], in_=zer_d[:])
        zer2 = cst.tile([1, 1024], bf16); nc.sync.dma_start(out=zer2[:], in_=zer2_d[:])
        nfT = cst.tile([32, N_WIN * WIN], bf16)
        nc.sync.dma_start(out=nfT[:], in_=nfT_d[:])

        def edge_tile(w, j, rin_w, nfs_w, wps):
            t = w * ET_PER_WIN + j
            e0 = t * P
            sh_t = sb3.tile([P, 10], f32, tag="sh")
            nc.sync.dma_start(out=sh_t[:], in_=shv_d[e0:e0 + P, :])
            S_t = sb3.tile([P, P], bf16, tag="S")
            nc.sync.dma_start(out=S_t[:], in_=sd_d[e0:e0 + P, :])

            rin_t = rin_w[:, j * P:(j + 1) * P]
            nfs_t = nfs_w[:, j * P:(j + 1) * P]
            pe1 = ppe1.tile([P, 512], f32, tag="pe1")
            # radial MLP (feature-major); cols: l1 [0:128], l2 [128:256],
            # tile2 [256:384], mxp [384:480]
            nc.tensor.matmul(out=pe1[0:64, 0:128], lhsT=rw1[:], rhs=rin_t, start=True, stop=True)
            f1 = sbf.tile([64, P], bf16, tag="f1")
            nc.scalar.activation(out=f1[:], in_=pe1[0:64, 0:128], func=AF.Silu, bias=rb1[:], scale=1.0)
            nc.tensor.matmul(out=pe1[0:64, 128:256], lhsT=rw2[:], rhs=f1[:], start=True, stop=True)
            f2 = sbf.tile([64, P], bf16, tag="f2")
            nc.scalar.activation(out=f2[:], in_=pe1[0:64, 128:256], func=AF.Silu, bias=rb2[:], scale=1.0)
            # tile2[p, e] = rfT[p%64, e]  (rw3 cols tiled 2x)
            nc.tensor.matmul(out=pe1[:, 256:384], lhsT=rw3t2[:], rhs=f2[:], start=True, stop=True)
            t2b = sbf.tile([P, P], bf16, tag="t2b")
            nc.scalar.copy(out=t2b[:], in_=pe1[:, 256:384])
            # hT (feature-major embedded source scalars)
            hTp = pshared.tile([32, P], f32, tag="scratch")
            nc.tensor.matmul(out=hTp[:], lhsT=embb[:], rhs=nfs_t, start=True, stop=True)
            hTb = sbf.tile([32, P], bf16, tag="hTb")
            nc.scalar.copy(out=hTb[:], in_=hTp[:])
            # monomials mT[(u,r), e] = h[2c+p//64, e] * rfT[p%64, e], 16 chunks
            mT = smT.tile([P, 2048], bf16, tag="mT")
            for qx in range(4):
                mq = pshared.tile([P, 512], f32, tag="scratch")
                for cc in range(4):
                    c = qx * 4 + cc
                    nc.tensor.matmul(out=mq[:, cc * P:(cc + 1) * P],
                                     lhsT=embrep[:, c * P:(c + 1) * P],
                                     rhs=nfs_t, start=True, stop=True)
                mqb = sbf.tile([P, 512], bf16, tag="mqb", bufs=3)
                nc.scalar.copy(out=mqb[:], in_=mq[:])
                nc.vector.tensor_tensor(
                    out=mT[:, qx * 512:(qx + 1) * 512].rearrange("p (c z) -> p c z", z=P),
                    in0=mqb[:].rearrange("p (c z) -> p c z", z=P),
                    in1=t2b[:][:, None, :].broadcast_to([P, 4, P]),
                    op=AL.mult)
            # mixed = mT @ A2 + hT @ B2  (PSUM accumulate), mxp = pe1[:,384:480]
            mxp = pe1[:, 384:480]
            for c in range(16):
                nc.tensor.matmul(out=mxp, lhsT=mT[:, c * P:(c + 1) * P],
                                 rhs=a2[:, c * 96:(c + 1) * 96],
                                 start=(c == 0), stop=False)
            nc.tensor.matmul(out=mxp, lhsT=hTb[:], rhs=b2[:], start=False, stop=True)
            # messages: msgs[e, LOFF+i*32+u] = sh[e, SOFF+i] * mixed[e, l*32+u]
            msgs = sb3.tile([P, HID + 1], bf16, tag="msgs")
            for l, d in enumerate(DIMS):
                nc.vector.tensor_tensor(
                    out=msgs[:, LOFF[l]:LOFF[l] + 32 * d].rearrange("p (i u) -> p i u", u=32),
                    in0=sh_t[:, SOFF[l]:SOFF[l] + d][:, :, None].broadcast_to([P, d, 32]),
                    in1=mxp[:, l * 32:(l + 1) * 32][:, None, :].broadcast_to([P, d, 32]),
                    op=AL.mult)
            nc.vector.tensor_copy(out=msgs[:, HID:HID + 1], in_=sh_t[:, 9:10])
            # scatter-accumulate into window psum
            nc.tensor.matmul(out=wps[:], lhsT=S_t[:], rhs=msgs[:],
                             start=(j == 0), stop=(j == ET_PER_WIN - 1))

        # J-columns present (all 9 irrep rows)
        JLIST = [(l, i) for l in range(3) for i in range(DIMS[l])]
        JIDX = {Ij: n for n, Ij in enumerate(JLIST)}

        def build_rep(xT2, I, blkcnt):
            """Replicated tile for irrep-row I from row-block-major xT2
            ([32, 9*128], block jn holds features cJ..cJ+32 on partitions
            0:32). Returns SBUF bf16 [128, 1024]: chunk kc col-block holds
            xT2-row (4kc + p//32) of block I at partition p."""
            jn = JIDX[I]
            pr = prep.tile([P, 1024], f32, tag="rep")
            for kc in range(8):
                nc.tensor.matmul(out=pr[:, kc * P:(kc + 1) * P],
                                 lhsT=eu8[0:32, kc * P:(kc + 1) * P],
                                 rhs=xT2[0:32, jn * P:(jn + 1) * P],
                                 start=True, stop=True)
            rep_sb = srep.tile([P, 1024], bf16, tag="repsb")
            nc.scalar.copy(out=rep_sb[:], in_=pr[:])
            return rep_sb

        def make_xT2(x_bf, tagp):
            """row-block transpose of x_bf [128, 288] -> [32, 9*128] bf16:
            block jn holds x_bf[:, cJ:cJ+32]^T on partitions 0:32."""
            tpa = pshared.tile([32, 5 * P], bf16, tag="scratch")
            for jn in range(5):
                (l, i) = JLIST[jn]
                cJ = LOFF[l] + i * 32
                nc.tensor.transpose(out=tpa[:, jn * P:(jn + 1) * P],
                                    in_=x_bf[:, cJ:cJ + 32], identity=identb[:])
            tpb = pshared.tile([32, 4 * P], bf16, tag="scratch")
            for jn in range(5, 9):
                (l, i) = JLIST[jn]
                cJ = LOFF[l] + i * 32
                nc.tensor.transpose(out=tpb[:, (jn - 5) * P:(jn - 4) * P],
                                    in_=x_bf[:, cJ:cJ + 32], identity=identb[:])
            xT2 = stv.tile([32, 9 * P], bf16, tag=tagp)
            nc.vector.tensor_copy(out=xT2[:, 0:5 * P], in_=tpa[:])
            nc.vector.tensor_copy(out=xT2[:, 5 * P:9 * P], in_=tpb[:])
            return xT2

        def make_aT(x_bf):
            """feature-major transpose of x_bf [128, 288] -> [128, 384] bf16
            (for the c1 block-diagonal matmuls)."""
            tp = pshared.tile([P, 384], bf16, tag="scratch")
            nc.tensor.transpose(out=tp[:, 0:P], in_=x_bf[:, 0:P], identity=identb[:])
            nc.tensor.transpose(out=tp[:, P:2 * P], in_=x_bf[:, P:2 * P], identity=identb[:])
            nc.tensor.transpose(out=tp[0:32, 2 * P:3 * P], in_=x_bf[:, 2 * P:HID], identity=identb[:])
            aT = stv.tile([P, 3 * P], bf16, tag="aT")
            nc.vector.tensor_copy(out=aT[:, 0:2 * P], in_=tp[:, 0:2 * P])
            nc.vector.tensor_copy(out=aT[0:32, 2 * P:3 * P], in_=tp[0:32, 2 * P:3 * P])
            return aT

        blk_counter = [0]

        def do_pass(blocks, emi_list, xT2, tile_v, qm_ps):
            # group blocks by I (list is sorted by (I, J))
            bi = 0
            nb = len(blocks)
            while bi < nb:
                I = blocks[bi][0]
                bj = bi
                while bj < nb and blocks[bj][0] == I:
                    bj += 1
                rep_sb = build_rep(xT2, I, bj - bi)
                for bk in range(bi, bj):
                    J = blocks[bk][1]
                    jn = JIDX[J]
                    PT = spt.tile([P, 1024], bf16, tag="PT")
                    eng = nc.gpsimd if blk_counter[0] % 4 == 3 else nc.vector
                    blk_counter[0] += 1
                    eng.tensor_tensor(
                        out=PT[:].rearrange("p (c z) -> p c z", z=P),
                        in0=rep_sb[:].rearrange("p (c z) -> p c z", z=P),
                        in1=tile_v[:, jn * P:(jn + 1) * P][:, None, :].broadcast_to([P, 8, P]),
                        op=AL.mult)
                    for kc in range(8):
                        for (c0, c1, off) in emi_list[bk][kc]:
                            nc.tensor.matmul(out=qm_ps[:, c0:c1],
                                             lhsT=PT[:, kc * P:(kc + 1) * P],
                                             rhs=omega[:, off:off + (c1 - c0)],
                                             start=False, stop=False,
                                             skip_group_check=True)
                bi = bj

        def node_window(w, wps):
            # normalize: a = wps[:, :288] / max(cnt, 1)
            cnt = sbf.tile([P, 1], f32, tag="cnt")
            nc.vector.tensor_scalar_max(out=cnt[:], in0=wps[:, HID:HID + 1], scalar1=1.0)
            rec = sbf.tile([P, 1], f32, tag="rec")
            nc.vector.reciprocal(out=rec[:], in_=cnt[:])
            a_bf = stv.tile([P, HID], bf16, tag="abf")
            nc.vector.tensor_scalar_mul(out=a_bf[:], in0=wps[:, :HID], scalar1=rec[:])
            aT = make_aT(a_bf)
            aT2 = make_xT2(a_bf, "aT2")
            # tile_v[:, jn*128:(jn+1)*128][p, z] = a[z, cJ + p%32]
            tile_v = stv.tile([P, 9 * P], bf16, tag="tv")
            for g in range(2):
                jlo = g * 5
                jhi = min(jlo + 5, 9)
                pv = prep.tile([P, 1024], f32, tag="rep")
                for jj in range(jlo, jhi):
                    nc.tensor.matmul(out=pv[:, (jj - jlo) * P:(jj - jlo + 1) * P],
                                     lhsT=t4q[0:32, :],
                                     rhs=aT2[0:32, jj * P:(jj + 1) * P],
                                     start=True, stop=True)
                nc.scalar.copy(out=tile_v[:, jlo * P:jhi * P],
                               in_=pv[:, 0:(jhi - jlo) * P])

            # qm layout: q at cols [0:288) (bank 0), msg at [512:800) (bank 1)
            MB = 512
            qm_ps = pqm.tile([P, 1024], f32, tag="qm")
            nc.tensor.matmul(out=qm_ps[:, 0:512], lhsT=zer[:, 0:P], rhs=zer2[:, 0:512], start=True, stop=False)
            nc.tensor.matmul(out=qm_ps[:, 512:1024], lhsT=zer[:, 0:P], rhs=zer2[:, 512:1024], start=True, stop=False)

            do_pass(plan.aa, plan.aa_emi, aT2, tile_v, qm_ps)
            # close bank-0 accumulation group, then evacuate q to bf16
            nc.tensor.matmul(out=qm_ps[:, 0:HID], lhsT=zer[:, 0:P], rhs=zer2[:, 0:HID],
                             start=False, stop=True)
            q_bf = stv.tile([P, HID], bf16, tag="qbf")
            nc.vector.tensor_copy(out=q_bf[:], in_=qm_ps[:, 0:HID])
            qT2 = make_xT2(q_bf, "qT2")
            do_pass(plan.qa, plan.qa_emi, qT2, tile_v, qm_ps)
            # c1: msg += aT-chunks @ omc1-chunks  (block-diagonal linear)
            nc.tensor.matmul(out=qm_ps[:, MB + 0:MB + 128], lhsT=aT[:, 0:P],
                             rhs=omc1[:, 0:128], start=False, stop=False,
                             skip_group_check=True)
            nc.tensor.matmul(out=qm_ps[:, MB + 128:MB + 256], lhsT=aT[:, P:2 * P],
                             rhs=omc1[:, 128:256], start=False, stop=False,
                             skip_group_check=True)
            nc.tensor.matmul(out=qm_ps[:, MB + 256:MB + 288], lhsT=aT[0:32, 2 * P:3 * P],
                             rhs=omc1[0:32, 256:288], start=False,
                             stop=False, skip_group_check=True)
            # self connection (l=0 cols)
            nc.tensor.matmul(out=qm_ps[:, MB:MB + 32], lhsT=nfT[:, w * P:(w + 1) * P],
                             rhs=omself[:], start=False, stop=False,
                             skip_group_check=True)
            # close bank-1 group
            nc.tensor.matmul(out=qm_ps[:, MB:MB + HID], lhsT=zer[:, 0:P], rhs=zer2[:, 0:HID],
                             start=False, stop=True)
            # write out (DMA cannot read PSUM; bounce via SBUF)
            out_sb = stv.tile([P, HID], f32, tag="outsb")
            nc.vector.tensor_copy(out=out_sb[:], in_=qm_ps[:, MB:MB + HID])
            nc.sync.dma_start(out=out_d[w * P:(w + 1) * P, :], in_=out_sb[:])

        # ---------------- main loop ----------------
        for w in range(N_WIN):
            rin_w = sbw.tile([24, ESLOT], bf16, tag="rinw")
            nc.sync.dma_start(out=rin_w[:], in_=rinT_d[:, w * ESLOT:(w + 1) * ESLOT])
            nfs_w = sbw.tile([32, ESLOT], bf16, tag="nfsw")
            nc.sync.dma_start(out=nfs_w[:], in_=nfsT_d[:, w * ESLOT:(w + 1) * ESLOT])
            wps = pwps.tile([P, HID + 1], f32, tag="wps")
            for j in range(ET_PER_WIN):
                edge_tile(w, j, rin_w, nfs_w, wps)
            node_window(w, wps)

    nc.finalize()
    return nc


def _get_nc(plan):
    if 'nc' not in _NC_CACHE:
        _NC_CACHE['nc'] = build_nc(plan)
    return _NC_CACHE['nc']


def kernel(**inputs):
    global LAST_RESULT
    from concourse.bass_utils import run_bass_kernel_spmd

    inp = {k: np.asarray(v) for k, v in inputs.items()}
    plan = build_plan()
    W = fold_weights(inp)
    om = pack_omega(plan, W)

    # A2 repacked for (u-major-2, r-minor-64) monomial chunks:
    # chunk c, row p  ->  A2 row (p%64)*32 + (2c + p//64)
    A2 = W['A2'].astype(np.float32)
    a2p = np.zeros((128, 16 * 96), np.float32)
    pidx = np.arange(128)
    for c in range(16):
        rows = (pidx % 64) * 32 + (2 * c + pidx // 64)
        a2p[:, c * 96:(c + 1) * 96] = A2[rows, :]
    # fold b3 into B2: mixed gets (rf0 + b3) x h terms; b3 (x) h part is linear in h
    B2 = W['B2'].astype(np.float64).copy()
    b3 = inp['r_b3'].astype(np.float64)
    for u in range(32):
        B2[u, :] += b3 @ A2[np.arange(RHID) * 32 + u, :].astype(np.float64)
    # omc1 packed [128, 288]
    omc1 = W['omc1']
    omc1p = np.zeros((128, HID), np.float32)
    omc1p[:, 0:128] = omc1[0:128, 0:128]
    omc1p[:, 128:256] = omc1[128:256, 128:256]
    omc1p[0:32, 256:288] = omc1[256:288, 256:288]

    identb = np.eye(128, dtype=np.float32).astype(BF)
    # eu8: quadrant-replicated row-replication selectors
    eu8 = np.zeros((128, 8 * 128), np.float32)
    for q in range(4):
        for kc in range(8):
            for p in range(128):
                eu8[32 * q + (4 * kc + p // 32) % 32, kc * 128 + p] = 1.0
    # t4q: quadrant-replicated 4x row tiling
    t4q = np.zeros((128, 128), np.float32)
    for q in range(4):
        for p in range(128):
            t4q[32 * q + p % 32, p] = 1.0
    # rw3 cols tiled 2x: tile2[p, e] = rfT[p%64, e]
    rw3 = W['r_w3'].astype(np.float64)
    rw3t2 = np.zeros((64, 128), np.float64)
    rw3t2[:, 0:64] = rw3
    rw3t2[:, 64:128] = rw3
    # emb columns replicated: embrep chunk c col p -> emb col (2c + p//64)
    emb = W['emb']
    embrep = np.zeros((32, 16 * 128), np.float64)
    for c in range(16):
        for p in range(128):
            embrep[:, c * 128 + p] = emb[:, 2 * c + p // 64]

    shared = dict(
        omega=om,
        a2=a2p.astype(BF), b2=B2.astype(np.float32).astype(BF),
        omc1=omc1p.astype(BF), omself=W['omself'].astype(np.float32).astype(BF),
        rw1=W['r_w1'].astype(BF), rw2=W['r_w2'].astype(BF),
        rw3t2=rw3t2.astype(np.float32).astype(BF),
        embrep=embrep.astype(np.float32).astype(BF),
        embb=emb.astype(np.float32).astype(BF),
        rb1=W['r_b1'].reshape(64, 1), rb2=W['r_b2'].reshape(64, 1),
        eu8=eu8.astype(BF), t4q=t4q.astype(BF), identb=identb,
        zer=np.zeros((1, 128), BF),
        zer2=np.zeros((1, 1024), BF),
    )
    cores = pack_edges(inp)
    in_maps = []
    for c in range(N_CORES):
        m = dict(shared)
        m.update(rinT=cores[c]['rinT'], nfsT=cores[c]['nfsT'],
                 shv=cores[c]['shv'], sd=cores[c]['Sd'], nfT=cores[c]['nfT'])
        in_maps.append(m)

    nc = _get_nc(plan)
    res = run_bass_kernel_spmd(nc, in_maps, core_ids=list(range(N_CORES)))
    LAST_RESULT = res
    # undo the per-core node->window binning permutation
    outs = [np.asarray(res.results[c]['out'])[cores[c]['pos']] for c in range(N_CORES)]
    out_kap = np.concatenate(outs, axis=0).astype(np.float32)
    return ref_from_kap(out_kap)


if __name__ == "__main__":
    plan = build_plan()
    print(f"aa blocks: {len(plan.aa)}  qa blocks: {len(plan.qa)}")
    print(f"omega cols: {plan.totc}  ({plan.totc * 128 * 2 / 1e6:.1f} MB bf16)")
    print(f"emissions per node-tile: {plan.n_emi}")
    if len(sys.argv) > 1 and sys.argv[1] == '--build':
        nc = build_nc(plan)
        print("build OK")
